# revision 48
# baseline (speedup 1.0000x reference)
"""Trainium2 Bass kernel: AnisotropicHomogeneousNN (raw-bass, manual sems).

Math per sample: solve sum_i e^{-2 r_i s} x_i^2 = 1 for s, then
out = MLP(x e^{-r s}) * e^{s}.  Solver: one log-Halley step at s=0
(s1 = 2 phi w / (2 + phi - phi H F/G^2), phi = ln F, w = F/G, with
F = sum x^2 e, G = sum 2r x^2 e, H = sum 4r^2 x^2 e) + NITER_FULL=1
full log-Newton iteration; matches the 30-iter reference to ~4.8e-3.

Distribution: pure data parallelism, batch split over 8 cores.

Layout/pipeline:
- Feature-major "2-stacked" bf16 tiles [128, 512]: partition p<64 =
  feature p of even sample of a pair, p>=64 = feature p-64 of odd
  sample; columns = pair index (tile t = pair-cols [512t, 512t+512)).
- Load: 16 DMA chunks [128, 1024] f32 (4KB contiguous lines, 4
  staging slots); PE transposes 128x128 f32 blocks into 4 psum slots
  (TPIH/SBPW halves); ACT copies+converts psum->XT bf16; DVE squares
  into A; iter0 F/G reduce (FGPB) and Halley H reduce (HP0) ride
  along per tile as one-hot matmul accumulation groups.
- Newton, pair-granular: PE one-hot bcast s (bf16 hi/lo SHL rows
  reconstruct ~f32) -> SBPW/TPIH psum; ACT exp(-2 r s) -> EM; DVE
  EM*A -> MM; PE one-hot reduce -> FGPB.  Boundaries on DVE (+ACT Ln).
- Final per tile t: PE bcast -> SBF half; ACT D=exp(-r s), ES=exp(s);
  Pool XS=XT*D (gpsimd is SBUF-only); PE mm1 (4 block-diag matmuls
  into 4 distinct f32 psum slots: HP0, HP1, TPIH halves; one-tile-lag
  relu gating keeps mm1 contiguous); relus ACT c0,c1 / DVE c2,c3;
  PE mm2 (accum into FGPB; b2 folded into OF); DVE OF=(PB+b2)*ES
  (scalar_tensor_tensor); PE transpose -> TPO; DVE bf16 OC copy;
  store via Pool SWDGE DMA with in-flight bf16->f32 cast (only the
  software DGE can cast).  bcast runs 2 tiles ahead; mm2 lags 2;
  xpose lags 3; OC lags 4 -- no stage waits on same-cycle results.
- PSUM banks (16KB): TPIH 2, SBPW 2, HP0 1, HP1 1, FGPB 1, TPO 1 = 8.
  FGPB serves as Newton F/G accumulator then mm2 accumulator; HP0 is
  the Halley H accumulator before becoming an mm1 slot.
- All DMA-completion waits use per-slot semaphores: DMA queue entries
  complete out of order, so cumulative DMA sem counts are ambiguous.

Raw Block style: standalone wait_ge instructions (walrus rejects >1
embedded sync wait per instruction).
"""

import numpy as np
import ml_dtypes

import concourse.bass as bass
import concourse.mybir as mybir
from concourse.bass_utils import run_bass_kernel_spmd

B, N, H, O = 262144, 64, 256, 64
NCORES = 8
BC = B // NCORES      # samples per core      32768
BP = BC // 2          # sample pairs per core 16384
T = 512               # pair-columns per tile
NT = BP // T          # tiles: 32
NP = NT // 2          # newton pairs-of-tiles: 16
NCH = 16              # input chunks
CW = 1024             # pair-cols per chunk
NSX = 4               # f32 staging slots
NITER_FULL = 1
NU = 1.0

f32 = mybir.dt.float32
bf16 = mybir.dt.bfloat16
AF = mybir.ActivationFunctionType
ALU = mybir.AluOpType

_last_exec_ns = None


def _host_consts(r, W1, b1, W2, b2):
    r = np.asarray(r, np.float32)
    W1 = np.asarray(W1, np.float32)
    b1 = np.asarray(b1, np.float32)
    W2 = np.asarray(W2, np.float32)
    b2 = np.asarray(b2, np.float32)

    # reduce lhsT [128, NT*128]: col = 128*t + m.  Rows of FG psum:
    # F_e -> m=t, F_o -> m=32+t, G_e -> m=64+t, G_o -> m=96+t
    RED = np.zeros((128, NT * 128), np.float32)
    for t in range(NT):
        RED[0:64, 128 * t + t] = 1.0
        RED[64:128, 128 * t + 32 + t] = 1.0
        RED[0:64, 128 * t + 64 + t] = 2.0 * r
        RED[64:128, 128 * t + 96 + t] = 2.0 * r

    # H-reduce (Halley): H = sum 4 r^2 x^2 e; rows match F layout
    RED2 = np.zeros((128, NT * 128), np.float32)
    for t in range(NT):
        RED2[0:64, 128 * t + t] = 4.0 * r * r
        RED2[64:128, 128 * t + 32 + t] = 4.0 * r * r

    # broadcast lhsT: per tile t, out row p<64 sums SHL rows {t, 64+t}
    # (hi_e+lo_e), p>=64 sums rows {32+t, 96+t} (hi_o+lo_o)
    BCT = np.zeros((128, NT * 128), np.float32)
    for t in range(NT):
        BCT[t, 128 * t + 0:128 * t + 64] = 1.0
        BCT[64 + t, 128 * t + 0:128 * t + 64] = 1.0
        BCT[32 + t, 128 * t + 64:128 * t + 128] = 1.0
        BCT[96 + t, 128 * t + 64:128 * t + 128] = 1.0

    W1BD = np.zeros((128, 4 * 128), np.float32)
    W2BD = np.zeros((128, 4 * 128), np.float32)
    for c in range(4):
        W1BD[0:64, 128 * c + 0:128 * c + 64] = W1[64 * c:64 * c + 64, :].T
        W1BD[64:128, 128 * c + 64:128 * c + 128] = W1[64 * c:64 * c + 64, :].T
        W2BD[0:64, 128 * c + 0:128 * c + 64] = W2[:, 64 * c:64 * c + 64].T
        W2BD[64:128, 128 * c + 64:128 * c + 128] = W2[:, 64 * c:64 * c + 64].T

    B1BD = np.zeros((128, 4), np.float32)
    for c in range(4):
        B1BD[0:64, c] = b1[64 * c:64 * c + 64]
        B1BD[64:128, c] = b1[64 * c:64 * c + 64]
    B2BD = np.zeros((128, 1), np.float32)
    B2BD[0:64, 0] = b2
    B2BD[64:128, 0] = b2

    RNEG2 = np.zeros((128, 1), np.float32)
    RNEG2[0:64, 0] = -2.0 * r
    RNEG2[64:128, 0] = -2.0 * r
    RNEG1 = 0.5 * RNEG2
    IDB = np.eye(128, dtype=np.float32)

    tobf = lambda a: a.astype(ml_dtypes.bfloat16)
    return {
        "RED": tobf(RED), "RED2": tobf(RED2), "BCT": tobf(BCT),
        "IDB": tobf(IDB), "IDB32": IDB,
        "W1BD": tobf(W1BD), "W2BD": tobf(W2BD),
        "B1BD": B1BD, "B2BD": B2BD, "RNEG2": RNEG2, "RNEG1": RNEG1,
    }


def _build():
    from contextlib import ExitStack
    nc = bass.Bass()

    x = nc.declare_dram_parameter("x", [BC, N], f32, isOutput=False)
    RED = nc.declare_dram_parameter("RED", [128, NT * 128], bf16, isOutput=False)
    RED2 = nc.declare_dram_parameter("RED2", [128, NT * 128], bf16, isOutput=False)
    BCT = nc.declare_dram_parameter("BCT", [128, NT * 128], bf16, isOutput=False)
    IDB = nc.declare_dram_parameter("IDB", [128, 128], bf16, isOutput=False)
    IDB32 = nc.declare_dram_parameter("IDB32", [128, 128], f32, isOutput=False)
    W1BD = nc.declare_dram_parameter("W1BD", [128, 512], bf16, isOutput=False)
    W2BD = nc.declare_dram_parameter("W2BD", [128, 512], bf16, isOutput=False)
    B1BD = nc.declare_dram_parameter("B1BD", [128, 4], f32, isOutput=False)
    B2BD = nc.declare_dram_parameter("B2BD", [128, 1], f32, isOutput=False)
    RNEG2 = nc.declare_dram_parameter("RNEG2", [128, 1], f32, isOutput=False)
    RNEG1 = nc.declare_dram_parameter("RNEG1", [128, 1], f32, isOutput=False)
    out = nc.declare_dram_parameter("out", [BC, N], f32, isOutput=True)

    # load view: chunk c = [128, (q f)], sample s = 2048c + 16p + q,
    # (q f) = 4KB contiguous per partition line
    xvc = x.rearrange("(c p q) f -> c p (q f)", p=128, q=16)
    # store view: tile t=(c,h): sample s = 2048c + 16p + 8h + 2jb + par
    ov = out.rearrange("(c p h jb par) f -> c h p jb (par f)",
                       p=128, h=2, jb=4, par=2)

    NCONST = 11
    CD = NCONST * 16

    es = ExitStack()
    with es:
        _n = [0]
        def sbuf(shape, dt):
            _n[0] += 1
            return es.enter_context(nc.sbuf_tensor(f"sb{_n[0]}", shape, dt))
        def psum(shape, dt):
            _n[0] += 1
            return es.enter_context(nc.psum_tensor(f"ps{_n[0]}", shape, dt))
        sem = lambda name: es.enter_context(nc.semaphore(name))

        # consts
        red = sbuf([128, NT * 128], bf16)
        red2 = sbuf([128, NT * 128], bf16)
        bct = sbuf([128, NT * 128], bf16)
        idb = sbuf([128, 128], bf16)
        idb32 = sbuf([128, 128], f32)
        w1 = sbuf([128, 512], bf16)
        w2 = sbuf([128, 512], bf16)
        b1t = sbuf([128, 4], f32)
        b2t = sbuf([128, 1], f32)
        rn2 = sbuf([128, 1], f32)
        rn1 = sbuf([128, 1], f32)
        # big tensors
        SX = [sbuf([128, CW], f32) for _ in range(NSX)]  # f32 staging
        XT = sbuf([128, BP], bf16)                       # feature-major x
        A = sbuf([128, BP], bf16)                        # x^2
        # newton
        S = sbuf([64, T], f32)
        LF = sbuf([64, T], f32)
        RG = sbuf([64, T], f32)
        P1 = sbuf([64, T], f32)
        Q1 = sbuf([64, T], f32)
        QS = sbuf([64, T], f32)
        Q2 = sbuf([64, T], f32)
        NM = sbuf([64, T], f32)
        DN = sbuf([64, T], f32)
        RD = sbuf([64, T], f32)
        PQ = sbuf([64, T], f32)
        SHL = sbuf([128, T], bf16)
        EM = [sbuf([128, CW], bf16) for _ in range(3)]
        MM = [sbuf([128, CW], bf16) for _ in range(3)]
        # final
        D = [sbuf([128, T], bf16) for _ in range(2)]
        ES = [sbuf([128, T], f32) for _ in range(4)]
        XS = [sbuf([128, T], bf16) for _ in range(2)]
        HR = [[sbuf([128, T], bf16) for _ in range(4)] for _ in range(3)]
        OF = [sbuf([128, T], bf16) for _ in range(2)]
        OC = [sbuf([128, T], bf16) for _ in range(2)]

        # PSUM: 8 banks exactly
        TPIH = psum([128, CW], f32)      # 2 banks: load xpose / newton / HP23
        SBPW = psum([128, CW], f32)      # 2 banks: newton bcast / final bcast
        HP0 = psum([128, T], f32)        # 1 bank
        HP1 = psum([128, T], f32)        # 1 bank
        FGPB = psum([128, T], f32)       # 1 bank: newton FG, final mm2 accum
        TPO = psum([128, T], bf16)       # 1 bank: out transposes
        NBUF = [SBPW, TPIH]              # newton bcast ping-pong
        HP4 = [HP0[:], HP1[:], TPIH[:, 0:T], TPIH[:, T:2 * T]]
        SBF = [SBPW[:, 0:T], SBPW[:, T:2 * T]]   # final bcast slots
        FGH = HP0                        # Halley H accum (newton only)
        XSL = [TPIH[:, 0:T], TPIH[:, T:2 * T],
               SBPW[:, 0:T], SBPW[:, T:2 * T]]   # load xpose slots

        s_cdma = sem("s_cdma")
        s_ldx = [sem(f"s_ldx{i}") for i in range(NSX)]
        s_pti = sem("s_pti")
        s_xt = sem("s_xt")
        s_a = sem("s_a")
        s_sb = sem("s_sb")
        s_e = sem("s_e")
        s_m = sem("s_m")
        s_red = sem("s_red")
        s_rh = sem("s_rh")
        s_ln = sem("s_ln")
        s_sml = sem("s_sml")
        s_q1 = sem("s_q1")
        s_nm = sem("s_nm")
        s_sbf = sem("s_sbf")
        s_d = sem("s_d")
        s_es = sem("s_es")
        s_xs = sem("s_xs")
        s_h = sem("s_h")
        s_ra = sem("s_ra")
        s_rv = sem("s_rv")
        s_op = sem("s_op")
        s_of = sem("s_of")
        s_pto = sem("s_pto")
        s_oc = sem("s_oc")
        s_st2 = [sem(f"s_st{i}") for i in range(2)]

        with nc.Block() as block:

            @block.sync
            def _(eng):
                for src_, dst in ((RED, red), (RED2, red2), (BCT, bct), (IDB, idb),
                                  (IDB32, idb32), (W1BD, w1), (W2BD, w2),
                                  (B1BD, b1t), (B2BD, b2t), (RNEG2, rn2),
                                  (RNEG1, rn1)):
                    eng.dma_start(out=dst[:], in_=src_[:]).then_inc(s_cdma, 16)
                for c in range(NCH):
                    if c >= NSX:
                        eng.wait_ge(s_pti, 8 * (c - NSX + 1))
                    eng.dma_start(out=SX[c % NSX][:], in_=xvc[c]) \
                       .then_inc(s_ldx[c % NSX], 16)


            @block.tensor
            def _(eng):
                eng.wait_ge(s_cdma, CD)
                # input transposes (f32, per half-chunk into TPIH slots)
                # + iter0 reduces (lag 1 chunk)
                def red0(t):
                    eng.wait_ge(s_a, t + 1)
                    eng.matmul(FGPB[:], red[:, 128 * t:128 * (t + 1)],
                               A[:, T * t:T * (t + 1)],
                               start=(t == 0), stop=(t == NT - 1)) \
                       .then_inc(s_red, 1)
                    eng.matmul(FGH[:], red2[:, 128 * t:128 * (t + 1)],
                               A[:, T * t:T * (t + 1)],
                               start=(t == 0), stop=(t == NT - 1)) \
                       .then_inc(s_rh, 1)
                for c in range(NCH):
                    for g2 in range(2):
                        g = 2 * c + g2
                        if g2 == 0:
                            eng.wait_ge(s_ldx[c % NSX], 16 * (c // NSX + 1))
                        if g >= 4:
                            eng.wait_ge(s_xt, g - 3)
                        for j in range(4):
                            eng.transpose(
                                XSL[g % 4][:, 128 * j:128 * (j + 1)],
                                SX[c % NSX][:, 128 * (4 * g2 + j):
                                            128 * (4 * g2 + j + 1)],
                                idb32[:]).then_inc(s_pti, 1)
                    if c >= 2:
                        for h in range(2):
                            red0(2 * (c - 2) + h)
                    if c == NCH - 1:
                        for h in range(2):
                            red0(2 * (c - 1) + h)
                for t in (NT - 2, NT - 1):
                    red0(t)
                # newton full iterations (pair-granular)
                for it in range(1, NITER_FULL + 1):
                    base_e = NP * (it - 1)
                    for u in range(NP + 2):
                        if u < NP:
                            if u < 2:
                                eng.wait_ge(s_sml, it)
                            else:
                                eng.wait_ge(s_e, base_e + u - 1)
                            for h in range(2):
                                t = 2 * u + h
                                ins = eng.matmul(
                                    NBUF[u % 2][:, T * h:T * (h + 1)],
                                    bct[:, 128 * t:128 * (t + 1)],
                                    SHL[:], start=True, stop=True)
                                if h == 1:
                                    ins.then_inc(s_sb, 1)
                        if u >= 2:
                            v = u - 2
                            eng.wait_ge(s_m, base_e + v + 1)
                            for h in range(2):
                                t = 2 * v + h
                                eng.matmul(FGPB[:],
                                           red[:, 128 * t:128 * (t + 1)],
                                           MM[v % 3][:, T * h:T * (h + 1)],
                                           start=(t == 0), stop=(t == NT - 1)) \
                                   .then_inc(s_red, 1)
                # final phase: bcast 2 ahead; xpose trails by 3
                for tb in range(2):
                    eng.wait_ge(s_sml, NITER_FULL + 1)
                    eng.matmul(SBF[tb % 2], bct[:, 128 * tb:128 * (tb + 1)],
                               SHL[:], start=True, stop=True) \
                       .then_inc(s_sbf, 1)
                for t in range(NT + 4):
                    tb = t + 2
                    if tb < NT:
                        eng.wait_ge(s_es, tb - 1)
                        eng.matmul(SBF[tb % 2], bct[:, 128 * tb:128 * (tb + 1)],
                                   SHL[:], start=True, stop=True) \
                           .then_inc(s_sbf, 1)
                    if t < NT:
                        # mm1 tile t into 4 distinct psum slots
                        eng.wait_ge(s_xs, t + 1)
                        for c in range(4):
                            if t >= 1:
                                if c == 0:
                                    eng.wait_ge(s_ra, 2 * t - 1)
                                elif c == 1:
                                    eng.wait_ge(s_ra, 2 * t)
                                elif c == 2:
                                    eng.wait_ge(s_rv, 2 * t - 1)
                                else:
                                    eng.wait_ge(s_rv, 2 * t)
                            eng.matmul(HP4[c], w1[:, 128 * c:128 * (c + 1)],
                                       XS[t % 2][:], start=True, stop=True) \
                               .then_inc(s_h, 1)
                    if 2 <= t <= NT + 1:
                        # mm2 tile t-2 into FGPB
                        v = t - 2
                        eng.wait_ge(s_ra, 2 * v + 2)
                        eng.wait_ge(s_rv, 2 * v + 2)
                        if v >= 1:
                            eng.wait_ge(s_of, v)
                        for c in range(4):
                            eng.matmul(FGPB[:], w2[:, 128 * c:128 * (c + 1)],
                                       HR[v % 3][c][:],
                                       start=(c == 0), stop=(c == 3)) \
                               .then_inc(s_op, 1 if c == 3 else 0)
                    if 3 <= t <= NT + 2:
                        # out transpose of tile t-3
                        v = t - 3
                        eng.wait_ge(s_of, v + 1)
                        if v >= 1:
                            eng.wait_ge(s_oc, v)
                        for jb in range(4):
                            ins = eng.transpose(
                                TPO[:, 128 * jb:128 * (jb + 1)],
                                OF[v % 2][:, 128 * jb:128 * (jb + 1)], idb[:])
                            if jb == 3:
                                ins.then_inc(s_pto, 1)

            @block.scalar
            def _(eng):
                eng.wait_ge(s_cdma, CD)
                # XT copies (f32 psum -> bf16 sbuf) per half-chunk
                for g in range(2 * NCH):
                    eng.wait_ge(s_pti, 4 * (g + 1))
                    eng.activation(XT[:, T * g:T * (g + 1)],
                                   XSL[g % 4],
                                   AF.Copy).then_inc(s_xt, 1)
                # newton exps + Ln per boundary
                eng.wait_ge(s_red, NT)
                eng.activation(LF[:], FGPB[0:64, :], AF.Ln).then_inc(s_ln, 1)
                for it in range(1, NITER_FULL + 1):
                    base_e = NP * (it - 1)
                    for u in range(NP):
                        eng.wait_ge(s_sb, base_e + u + 1)
                        if u >= 3:
                            eng.wait_ge(s_m, base_e + u - 2)
                        eng.activation(EM[u % 3][:], NBUF[u % 2][:], AF.Exp,
                                       scale=rn2[:, 0:1]).then_inc(s_e, 1)
                    eng.wait_ge(s_red, NT * (it + 1))
                    eng.activation(LF[:], FGPB[0:64, :], AF.Ln).then_inc(s_ln, 1)
                # final: relus c=0,1 then D/ES one tile ahead
                eng.wait_ge(s_sbf, 1)
                eng.activation(D[0][:], SBF[0], AF.Exp,
                               scale=rn1[:, 0:1]).then_inc(s_d, 1)
                eng.activation(ES[0][:], SBF[0], AF.Exp,
                               scale=NU).then_inc(s_es, 1)
                for t in range(NT + 1):
                    if 1 <= t <= NT:
                        v = t - 1
                        eng.wait_ge(s_h, 4 * v + 1)
                        if v >= 3:
                            eng.wait_ge(s_op, v - 2)
                        eng.activation(HR[v % 3][0][:], HP0[:], AF.Relu,
                                       bias=b1t[:, 0:1]).then_inc(s_ra, 1)
                        eng.wait_ge(s_h, 4 * v + 2)
                        eng.activation(HR[v % 3][1][:], HP1[:], AF.Relu,
                                       bias=b1t[:, 1:2]).then_inc(s_ra, 1)
                    e = t + 1
                    if e < NT:
                        eng.wait_ge(s_sbf, e + 1)
                        if e >= 2:
                            eng.wait_ge(s_xs, e - 1)
                        eng.activation(D[e % 2][:], SBF[e % 2], AF.Exp,
                                       scale=rn1[:, 0:1]).then_inc(s_d, 1)
                        if e >= 4:
                            eng.wait_ge(s_of, e - 3)
                        eng.activation(ES[e % 4][:], SBF[e % 2], AF.Exp,
                                       scale=NU).then_inc(s_es, 1)

            @block.vector
            def _(eng):
                eng.wait_ge(s_cdma, CD)
                # squares per half-chunk
                for g in range(2 * NCH):
                    eng.wait_ge(s_xt, g + 1)
                    eng.tensor_tensor(A[:, T * g:T * (g + 1)],
                                      XT[:, T * g:T * (g + 1)],
                                      XT[:, T * g:T * (g + 1)],
                                      ALU.mult).then_inc(s_a, 1)
                # newton chains + m-mults
                for it in range(NITER_FULL + 1):
                    eng.wait_ge(s_red, NT * (it + 1))
                    if it == 0:
                        # Halley step at s=0 on phi = ln F, w = F/G:
                        # s1 = 2 phi w / (2 + phi - phi H F / G^2)
                        eng.wait_ge(s_rh, NT)
                        eng.reciprocal(RG[:], FGPB[64:128, :])
                        eng.tensor_tensor(Q1[:], FGPB[0:64, :], RG[:],
                                          ALU.mult).then_inc(s_q1, 1)
                        eng.tensor_tensor(QS[:], FGH[0:64, :], RG[:], ALU.mult)
                        eng.tensor_tensor(QS[:], QS[:], Q1[:], ALU.mult)
                        eng.wait_ge(s_ln, 1)
                        eng.tensor_tensor(PQ[:], LF[:], QS[:], ALU.mult)
                        eng.tensor_tensor(DN[:], LF[:], PQ[:], ALU.subtract)
                        eng.tensor_scalar(DN[:], DN[:], 2.0, None, ALU.add)
                        eng.reciprocal(RD[:], DN[:])
                        eng.wait_ge(s_nm, 1)
                        eng.tensor_tensor(S[:], NM[:], RD[:], ALU.mult)
                    else:
                        eng.reciprocal(RG[:], FGPB[64:128, :])
                        eng.wait_ge(s_ln, it + 1)
                        eng.tensor_tensor(P1[:], LF[:], FGPB[0:64, :], ALU.mult)
                        eng.tensor_tensor(P1[:], P1[:], RG[:], ALU.mult)
                        eng.tensor_tensor(S[:], S[:], P1[:], ALU.add)
                    eng.tensor_scalar(SHL[0:64, :], S[:], 1.0, None, ALU.mult)
                    eng.tensor_tensor(SHL[64:128, :], S[:], SHL[0:64, :],
                                      ALU.subtract).then_inc(s_sml, 1)
                    if it == NITER_FULL:
                        break
                    base_e = NP * it
                    for u in range(NP):
                        eng.wait_ge(s_e, base_e + u + 1)
                        if u >= 3:
                            eng.wait_ge(s_red, NT * (it + 1) + 2 * u - 4)
                        eng.tensor_tensor(MM[u % 3][:], EM[u % 3][:],
                                          A[:, CW * u:CW * (u + 1)],
                                          ALU.mult).then_inc(s_m, 1)
                # final: relus c=2,3, OF (lag 3), OC (lag 4)
                for t in range(NT + 4):
                    if 1 <= t <= NT:
                        v = t - 1
                        eng.wait_ge(s_h, 4 * v + 3)
                        if v >= 3:
                            eng.wait_ge(s_op, v - 2)
                        eng.tensor_scalar(HR[v % 3][2][:], TPIH[:, 0:T],
                                          b1t[:, 2:3], 0.0, ALU.add, ALU.max) \
                           .then_inc(s_rv, 1)
                        eng.wait_ge(s_h, 4 * v + 4)
                        eng.tensor_scalar(HR[v % 3][3][:], TPIH[:, T:2 * T],
                                          b1t[:, 3:4], 0.0, ALU.add, ALU.max) \
                           .then_inc(s_rv, 1)
                    if 3 <= t <= NT + 2:
                        v = t - 3
                        eng.wait_ge(s_op, v + 1)
                        if v >= 2:
                            eng.wait_ge(s_pto, v - 1)
                        eng.scalar_tensor_tensor(OF[v % 2][:], FGPB[:],
                                                 b2t[:, 0:1], ES[v % 4][:],
                                                 ALU.add, ALU.mult) \
                           .then_inc(s_of, 1)
                    if t >= 4:
                        v = t - 4
                        eng.wait_ge(s_pto, v + 1)
                        if v >= 2:
                            eng.wait_ge(s_st2[v % 2], 16 * (v // 2))
                        eng.tensor_copy(OC[v % 2][:], TPO[:]).then_inc(s_oc, 1)

            @block.gpsimd
            def _(eng):
                eng.wait_ge(s_cdma, CD)
                # Halley NM branch (off DVE's critical chain)
                eng.wait_ge(s_ln, 1)
                eng.wait_ge(s_q1, 1)
                eng.tensor_tensor(NM[:], LF[:], Q1[:], ALU.mult)
                eng.tensor_scalar(NM[:], NM[:], 2.0, None, ALU.mult) \
                   .then_inc(s_nm, 1)
                # final: XS multiplies + casting SWDGE stores (bf16->f32)
                for t in range(NT + 5):
                    if t < NT:
                        eng.wait_ge(s_d, t + 1)
                        if t >= 2:
                            eng.wait_ge(s_h, 4 * (t - 1))
                        eng.tensor_tensor(XS[t % 2][:], XT[:, T * t:T * (t + 1)],
                                          D[t % 2][:], ALU.mult) \
                           .then_inc(s_xs, 1)
                    if t >= 5:
                        v = t - 5
                        eng.wait_ge(s_oc, v + 1)
                        eng.dma_start(
                            out=ov[v // 2][v % 2],
                            in_=OC[v % 2][:].rearrange("p (jb pf) -> p jb pf",
                                                       jb=4)) \
                           .then_inc(s_st2[v % 2], 16)

    return nc


_cached = None


def kernel(x, r, W1, b1, W2, b2, _trace=False):
    global _cached, _last_exec_ns
    if _cached is None:
        _cached = _build()
    nc = _cached
    consts = _host_consts(r, W1, b1, W2, b2)
    x = np.ascontiguousarray(np.asarray(x, np.float32))
    in_maps = []
    for i in range(NCORES):
        m = {"x": x[i * BC:(i + 1) * BC]}
        m.update(consts)
        in_maps.append(m)
    res = run_bass_kernel_spmd(nc, in_maps, list(range(NCORES)),
                               trace=_trace)
    _last_exec_ns = res.exec_time_ns
    return np.concatenate([res.results[i]["out"] for i in range(NCORES)],
                          axis=0)
